# revision 24
# baseline (speedup 1.0000x reference)
"""Trainium2 Bass kernel for nn_Attn_90623809945974.

out[b, 0, :] = softmax_l( hidden[0,b,:] . (W @ enc[l,b,:] + bias) )
             = softmax_l( (W^T h_b) . enc[l,b,:] )   (bias const per b -> cancels)

Sharding: data-parallel over batch (B=64 -> 8 per core); W replicated.

v5 design (fp32-exact, DMA-bound; ~198.7us vs 224.3us baseline):
  - DMA is a single serial 360 GB/s resource in the cost model, so the
    64 MiB/core enc stream (186.4us) dominates: issue the first enc tile
    DMA BEFORE W/hid so the stream owns the DMA engines from ~2.4us, keep
    it gap-free (8x 2MB tile buffers), and slice the final 5 tiles into
    per-batch DMAs so tail compute tracks arrivals at slice granularity.
  - Per 128-l tile: 8x DVE fused multiply+reduce (scalar_tensor_tensor,
    594ns each, < the 728ns slice arrival rate) -> eacc [128,8]; one PE
    transpose -> PSUM [8,128]; ACT evacuates into esm.
  - Last 4 tiles: PE takes b0..b2 via the masked-vm matmul path, DVE keeps
    the late-arriving b3..b7 (shortest per-unit chain after the last byte).
  - Softmax: 7 big 512-l chunks each shifted by their own max (m_c, s_c
    stored, independent -> no running-max serialization); the final 4
    tile-chunks reuse m_prev = max(m_0..m_6) as shift (known before they
    land, overflow-safe), with the PSUM evacuation FUSED into the ACT
    exp+accum. Big slices are pre-multiplied by exp(m_c - m_prev) on idle
    DVE slack during the tail tiles, so after S = sum s_c*f_c the only
    remaining factor is 1/S: one multiply split across DVE/ACT/Pool
    regions (Pool via normalize_recip - TensorScalarPtr is not legal on
    Pool on real HW), then a single output DMA.
  - Tail after the last byte ~6.9us: 900 DMA-sem + 594 stt + transpose +
    fused exp/accum + S-combine + ~1.2us rescale + trigger/dge/transfer/
    sem/drain protocol (~3us, irreducible).
"""
import numpy as np

import concourse.bass as bass
import concourse.bacc as bacc
import concourse.mybir as mybir
from concourse import tile, masks
from concourse.bass_utils import run_bass_kernel_spmd

L = 4096
B = 64
H = 512
NCORES = 8
BL = B // NCORES   # 8
LT = 128           # l-tile rows
LC = 512           # big softmax chunk
F32 = mybir.dt.float32
A = mybir.AluOpType

_cache = {}


def _build(L=L, num_devices=NCORES, pe_chunks=None, do_compile=True):
    NT = L // LT                     # 32 l-tiles
    NCH = L // LC                    # 8
    TPC = LC // LT                   # 4
    NBIG = NCH - 1                   # 7 big chunks
    SLICED = set(range(NT - 5, NT))  # per-b DMA slicing
    PEB = {t: [0, 1, 2] for t in range(NT - 4, NT)}  # PE-assist batches
    nc = bacc.Bacc("TRN2", target_bir_lowering=False, debug=False, num_devices=num_devices)
    enc_d = nc.dram_tensor("enc", [L, BL * H], F32, kind="ExternalInput").ap()
    hid_d = nc.dram_tensor("hid", [BL, H], F32, kind="ExternalInput").ap()
    w_d = nc.dram_tensor("w", [H, H], F32, kind="ExternalInput").ap()
    out_d = nc.dram_tensor("out", [BL, L], F32, kind="ExternalOutput").ap()

    with tile.TileContext(nc) as tc:
        with (
            tc.tile_pool(name="const", bufs=1) as constp,
            tc.tile_pool(name="keep", bufs=1) as keepp,
            tc.tile_pool(name="tiles", bufs=8) as tilep,
            tc.tile_pool(name="eacc", bufs=4) as eaccp,
            tc.tile_pool(name="epsum", bufs=2, space="PSUM") as epsum,
            tc.tile_pool(name="tpsum", bufs=2, space="PSUM") as tpsum,
            tc.tile_pool(name="enct", bufs=3) as enctp,
            tc.tile_pool(name="smt", bufs=2) as smtp,
            tc.tile_pool(name="post", bufs=1) as postp,
        ):
            # softmax chunks: 7x512 then 4x128 (short tail chunks)
            SCH = [(i * LC, LC) for i in range(NBIG)]
            SCH += [(NBIG * LC + i * LT, LT) for i in range(TPC)]
            NSC = len(SCH)
            tile_ends_chunk = {}
            for ci, (s0, sz) in enumerate(SCH):
                tile_ends_chunk[(s0 + sz) // LT - 1] = ci

            ident = constp.tile([128, 128], F32, tag="ident")
            vb = keepp.tile([128, BL * H], F32, tag="vb")
            vm = keepp.tile([128, 12 * BL], F32, tag="vm")   # masked V, b0..2 x j
            esm = keepp.tile([BL, L], F32, tag="esm")
            mstore = keepp.tile([BL, NBIG], F32, tag="mstore")
            sstore = keepp.tile([BL, NSC], F32, tag="sstore")
            negmp = keepp.tile([BL, 1], F32, tag="negmp")    # -max(m_0..m_6)
            ffac = keepp.tile([BL, NSC], F32, tag="ffac")

            # ---------- DMA issue helper (enc tiles stream in order) --------
            enc_tiles = {}

            def issue_tile_dma(t):
                tt = tilep.tile([LT, BL * H], F32, tag="enc_t")
                if t in SLICED:
                    for b in range(BL):
                        nc.sync.dma_start(
                            tt[:, b * H:(b + 1) * H],
                            enc_d[t * LT:(t + 1) * LT, b * H:(b + 1) * H],
                        )
                else:
                    nc.sync.dma_start(tt[:], enc_d[t * LT:(t + 1) * LT, :])
                enc_tiles[t] = tt

            # enc tile 0 first: the stream owns the DMA engines from t~2.4us
            issue_tile_dma(0)

            with (
                tc.tile_pool(name="pre", bufs=1) as prep,
                tc.tile_pool(name="prepsum", bufs=1, space="PSUM") as prepsum,
            ):
                # W + hid right after the first enc tile
                hid_sb = prep.tile([BL, H], F32, tag="hid_sb")
                nc.sync.dma_start(hid_sb[:], hid_d[:])
                w_sb = prep.tile([128, 4 * H], F32, tag="w_sb")
                for j in range(4):
                    nc.sync.dma_start(w_sb[:, j * H:(j + 1) * H], w_d[j * 128:(j + 1) * 128, :])
                issue_tile_dma(1)
                issue_tile_dma(2)

                masks.make_identity(nc, ident[:])

                # h8t[p, j*8+b] = hid[b, j*128+p]
                h8t = prep.tile([128, 4 * BL], F32, tag="h8t")
                for j in range(4):
                    ps = prepsum.tile([128, BL], F32, tag="pre_ps")
                    nc.tensor.transpose(ps[:], hid_sb[:, j * 128:(j + 1) * 128], ident[:BL, :BL])
                    nc.vector.tensor_copy(h8t[:, j * BL:(j + 1) * BL], ps[:])

                # v8[o, j*8+b] = v[b, j*128+o],  v_b = W^T h_b
                v8 = prep.tile([128, 4 * BL], F32, tag="v8")
                for j in range(4):
                    psv = prepsum.tile([128, BL], F32, tag="pre_ps")
                    for i in range(4):
                        nc.tensor.matmul(
                            psv[:],
                            w_sb[:, i * H + j * 128: i * H + (j + 1) * 128],
                            h8t[:, i * BL:(i + 1) * BL],
                            start=(i == 0), stop=(i == 3),
                        )
                    nc.vector.tensor_copy(v8[:, j * BL:(j + 1) * BL], psv[:])

                # vt[b, j*128+o] = v[b, j*128+o]
                psv8 = prepsum.tile([BL, H], F32, tag="pre_big")
                for j in range(4):
                    nc.tensor.transpose(
                        psv8[:, j * 128:(j + 1) * 128],
                        v8[:, j * BL:(j + 1) * BL],
                        ident[:],
                    )
                vt = prep.tile([BL, H], F32, tag="vt")
                nc.vector.tensor_copy(vt[:], psv8[:])
                # flatten to partition 0 (GPSIMD ISA inputs must start there)
                vtf = prep.tile([1, BL * H], F32, tag="vtf")
                nc.sync.dma_start(
                    vtf[:].rearrange("p (b h) -> p b h", b=BL), vt[:])

                # vb[p, b*H+h] = v[b, h] for all p (Pool partition broadcast)
                for b in range(BL):
                    nc.gpsimd.partition_broadcast(
                        vb[:, b * H:(b + 1) * H], vtf[:, b * H:(b + 1) * H])

                # masked V blocks for the PE-assist path: block (b,j) is
                # [128,8] with column b = v8[:, j*8+b], rest zero.
                nc.vector.memset(vm[:], 0.0)
                for b in range(3):
                    for j in range(4):
                        blk = (b * 4 + j) * BL
                        nc.vector.tensor_copy(
                            vm[:, blk + b: blk + b + 1],
                            v8[:, j * BL + b: j * BL + b + 1],
                        )

            # ---------- main tile loop ----------
            next_issue = [3]
            premult = []  # big chunks awaiting their exp(m_c - m_prev) factor
            for t in range(NT):
                while next_issue[0] < min(t + 8, NT):
                    issue_tile_dma(next_issue[0])
                    next_issue[0] += 1
                # fold the combine factor into big slices on idle DVE slack
                for _ in range(2):
                    if t >= NT - 4 and premult:
                        cb = premult.pop(0)
                        nc.vector.tensor_scalar_mul(
                            esm[:, cb * LC:(cb + 1) * LC],
                            esm[:, cb * LC:(cb + 1) * LC],
                            ffac[:, cb:cb + 1])
                tt = enc_tiles.pop(t)
                peb = PEB.get(t, [])
                dvb = [b for b in range(BL) if b not in peb]

                eacc = eaccp.tile([LT, BL], F32, tag="eacc")
                if peb:
                    nc.gpsimd.memset(eacc[:, 0:len(peb)], 0.0)
                for b in dvb:
                    nc.vector.scalar_tensor_tensor(
                        out=tt[:, b * H:(b + 1) * H],
                        in0=tt[:, b * H:(b + 1) * H],
                        scalar=1.0,
                        in1=vb[:, b * H:(b + 1) * H],
                        op0=A.mult,
                        op1=A.mult,
                        accum_out=eacc[:, b:b + 1],
                    )
                pe_t = epsum.tile([BL, LT], F32, tag="pe_t")
                if peb:
                    # PE path for b0..2: transpose enc blocks, masked matmuls
                    # accumulate energies into pe_t rows 0..2; the eacc
                    # transpose adds rows 3..7 (cols 0..2 zeroed above).
                    encts = {}
                    for b in peb:
                        tp = tpsum.tile([128, H], F32, tag="tps")
                        for j in range(4):
                            nc.tensor.transpose(
                                tp[:, j * 128:(j + 1) * 128],
                                tt[:, b * H + j * 128: b * H + (j + 1) * 128],
                                ident[:],
                            )
                        enct = enctp.tile([128, H], F32, tag="enct")
                        nc.scalar.copy(enct[:], tp[:])
                        encts[b] = enct
                    first = True
                    for b in peb:
                        for j in range(4):
                            blk = (b * 4 + j) * BL
                            nc.tensor.matmul(
                                pe_t[:],
                                vm[:, blk:blk + BL],
                                encts[b][:, j * 128:(j + 1) * 128],
                                start=first, stop=False,
                            )
                            first = False
                    nc.tensor.matmul(
                        pe_t[:], eacc[:], ident[:],
                        is_transpose=True, start=False, stop=True)
                else:
                    nc.tensor.transpose(pe_t[:], eacc[:], ident[:])
                base = t * LT
                ci = tile_ends_chunk.get(t)
                if ci is not None and ci >= NBIG:
                    # tail tile-chunk: fused PSUM-evac + exp(x - m_prev),
                    # accumulating s_c — one ACT op instead of copy+exp
                    nc.scalar.activation(
                        out=esm[:, base:base + LT], in_=pe_t[:],
                        func=mybir.ActivationFunctionType.Exp,
                        bias=negmp[:], scale=1.0, accum_out=sstore[:, ci:ci + 1])
                    continue
                nc.scalar.copy(esm[:, base:base + LT], pe_t[:])

                if ci is None:
                    continue
                sl = esm[:, SCH[ci][0]:SCH[ci][0] + SCH[ci][1]]
                if ci < NBIG:
                    # big chunk: shift by own max
                    mx_c = smtp.tile([BL, 1], F32, tag=f"mx_{ci % 2}")
                    nc.vector.tensor_reduce(
                        out=mx_c[:], in_=sl, axis=mybir.AxisListType.X, op=A.max)
                    nc.vector.tensor_copy(mstore[:, ci:ci + 1], mx_c[:])
                    negm = smtp.tile([BL, 1], F32, tag=f"ng_{ci % 2}")
                    nc.vector.tensor_scalar_mul(negm[:], mx_c[:], -1.0)
                    nc.scalar.activation(
                        out=sl, in_=sl,
                        func=mybir.ActivationFunctionType.Exp,
                        bias=negm[:], scale=1.0, accum_out=sstore[:, ci:ci + 1])
                    if ci == NBIG - 1:
                        # m_prev = max over big-chunk maxes; precompute the
                        # big-chunk combine factors exp(m_c - m_prev)
                        nc.vector.tensor_reduce(
                            out=negmp[:], in_=mstore[:],
                            axis=mybir.AxisListType.X, op=A.max)
                        nc.vector.tensor_scalar_mul(negmp[:], negmp[:], -1.0)
                        nc.vector.memset(ffac[:, NBIG:], 1.0)
                        nc.scalar.activation(
                            out=ffac[:, 0:NBIG], in_=mstore[:],
                            func=mybir.ActivationFunctionType.Exp,
                            bias=negmp[:], scale=1.0)
                        premult.extend(range(NBIG))

            # ---------- epilogue: S = sum_c s_c * ffac_c; x 1/S; store ------
            # big slices already carry exp(m_c - m_prev); the only remaining
            # factor is the global 1/S, split into three engine regions. The
            # DVE/ACT regions cover big chunks (written long before), so they
            # are gated only by 1/S, not by the last tile's exp.
            ssum = postp.tile([BL, 1], F32, tag="ssum")
            ssum2 = postp.tile([BL, 1], F32, tag="ssum2")
            rsum = postp.tile([BL, 1], F32, tag="rsum")
            wfac = postp.tile([BL, NSC], F32, tag="wfac")
            nc.vector.scalar_tensor_tensor(
                out=wfac[:], in0=ffac[:], scalar=1.0, in1=sstore[:],
                op0=A.mult, op1=A.mult, accum_out=ssum[:])
            nc.vector.tensor_copy(ssum2[:], ssum[:])
            nc.vector.reciprocal(rsum[:], ssum[:])

            # x 1/S in three engine regions (Pool divides internally)
            D0, D1 = 2240, 3328
            nc.vector.tensor_scalar_mul(esm[:, 0:D0], esm[:, 0:D0], rsum[:])
            nc.scalar.activation(
                out=esm[:, D0:D1], in_=esm[:, D0:D1],
                func=mybir.ActivationFunctionType.Copy, scale=rsum[:])
            nc.gpsimd.normalize_recip(esm[:, D1:], esm[:, D1:], ssum2[:])
            nc.sync.dma_start(out_d[:], esm[:])

    if do_compile:
        nc.compile()
    return nc


def kernel(hidden, encoder_outputs, W, b):
    hidden = np.asarray(hidden, dtype=np.float32)
    enc = np.asarray(encoder_outputs, dtype=np.float32)
    W = np.asarray(W, dtype=np.float32)

    if "nc" not in _cache:
        _cache["nc"] = _build()
    nc = _cache["nc"]

    in_maps = []
    for c in range(NCORES):
        b0 = c * BL
        in_maps.append({
            "enc": np.ascontiguousarray(enc[:, b0:b0 + BL, :]).reshape(L, BL * H),
            "hid": np.ascontiguousarray(hidden[0, b0:b0 + BL, :]),
            "w": W,
        })
    res = run_bass_kernel_spmd(nc, in_maps, core_ids=list(range(NCORES)))
    out = np.empty((B, 1, L), dtype=np.float32)
    for c in range(NCORES):
        out[c * BL:(c + 1) * BL, 0, :] = res.results[c]["out"]
    return out


# revision 25
# speedup vs baseline: 1.0002x; 1.0002x over previous
"""Trainium2 Bass kernel for nn_Attn_90623809945974.

out[b, 0, :] = softmax_l( hidden[0,b,:] . (W @ enc[l,b,:] + bias) )
             = softmax_l( (W^T h_b) . enc[l,b,:] )   (bias const per b -> cancels)

Sharding: data-parallel over batch (B=64 -> 8 per core); W replicated.

v5 design (fp32-exact, DMA-bound; ~198.7us vs 224.3us baseline):
  - DMA is a single serial 360 GB/s resource in the cost model, so the
    64 MiB/core enc stream (186.4us) dominates: issue the first enc tile
    DMA BEFORE W/hid so the stream owns the DMA engines from ~2.4us, keep
    it gap-free (8x 2MB tile buffers), and slice the final 5 tiles into
    per-batch DMAs so tail compute tracks arrivals at slice granularity.
  - Per 128-l tile: 8x DVE fused multiply+reduce (scalar_tensor_tensor,
    594ns each, < the 728ns slice arrival rate) -> eacc [128,8]; one PE
    transpose -> PSUM [8,128]; ACT evacuates into esm.
  - Last 4 tiles: PE takes b0..b2 via the masked-vm matmul path, DVE keeps
    the late-arriving b3..b7 (shortest per-unit chain after the last byte).
  - Softmax: 7 big 512-l chunks each shifted by their own max (m_c, s_c
    stored, independent -> no running-max serialization); the final 4
    tile-chunks reuse m_prev = max(m_0..m_6) as shift (known before they
    land, overflow-safe), with the PSUM evacuation FUSED into the ACT
    exp+accum. Big slices are pre-multiplied by exp(m_c - m_prev) on idle
    DVE slack during the tail tiles, so after S = sum s_c*f_c the only
    remaining factor is 1/S: one multiply split across DVE/ACT/Pool
    regions (Pool via normalize_recip - TensorScalarPtr is not legal on
    Pool on real HW), then a single output DMA.
  - Tail after the last byte ~6.9us: 900 DMA-sem + 594 stt + transpose +
    fused exp/accum + S-combine + ~1.2us rescale + trigger/dge/transfer/
    sem/drain protocol (~3us, irreducible).
"""
import numpy as np

import concourse.bass as bass
import concourse.bacc as bacc
import concourse.mybir as mybir
from concourse import tile, masks
from concourse.bass_utils import run_bass_kernel_spmd

L = 4096
B = 64
H = 512
NCORES = 8
BL = B // NCORES   # 8
LT = 128           # l-tile rows
LC = 512           # big softmax chunk
F32 = mybir.dt.float32
A = mybir.AluOpType

_cache = {}


def _build(L=L, num_devices=NCORES, pe_chunks=None, do_compile=True):
    NT = L // LT                     # 32 l-tiles
    NCH = L // LC                    # 8
    TPC = LC // LT                   # 4
    NBIG = NCH - 1                   # 7 big chunks
    SLICED = set(range(NT - 5, NT))  # per-b DMA slicing
    PEB = {t: [0, 1, 2] for t in range(NT - 4, NT)}  # PE-assist batches
    nc = bacc.Bacc("TRN2", target_bir_lowering=False, debug=False, num_devices=num_devices)
    enc_d = nc.dram_tensor("enc", [L, BL * H], F32, kind="ExternalInput").ap()
    hid_d = nc.dram_tensor("hid", [BL, H], F32, kind="ExternalInput").ap()
    w_d = nc.dram_tensor("w", [H, H], F32, kind="ExternalInput").ap()
    out_d = nc.dram_tensor("out", [BL, L], F32, kind="ExternalOutput").ap()

    with tile.TileContext(nc) as tc:
        with (
            tc.tile_pool(name="const", bufs=1) as constp,
            tc.tile_pool(name="keep", bufs=1) as keepp,
            tc.tile_pool(name="tiles", bufs=8) as tilep,
            tc.tile_pool(name="eacc", bufs=4) as eaccp,
            tc.tile_pool(name="epsum", bufs=2, space="PSUM") as epsum,
            tc.tile_pool(name="tpsum", bufs=2, space="PSUM") as tpsum,
            tc.tile_pool(name="enct", bufs=3) as enctp,
            tc.tile_pool(name="smt", bufs=2) as smtp,
            tc.tile_pool(name="post", bufs=1) as postp,
        ):
            # softmax chunks: 7x512 then 4x128 (short tail chunks)
            SCH = [(i * LC, LC) for i in range(NBIG)]
            SCH += [(NBIG * LC + i * LT, LT) for i in range(TPC)]
            NSC = len(SCH)
            tile_ends_chunk = {}
            for ci, (s0, sz) in enumerate(SCH):
                tile_ends_chunk[(s0 + sz) // LT - 1] = ci

            ident = constp.tile([128, 128], F32, tag="ident")
            vb = keepp.tile([128, BL * H], F32, tag="vb")
            vm = keepp.tile([128, 12 * BL], F32, tag="vm")   # masked V, b0..2 x j
            esm = keepp.tile([BL, L], F32, tag="esm")
            mstore = keepp.tile([BL, NBIG], F32, tag="mstore")
            sstore = keepp.tile([BL, NSC], F32, tag="sstore")
            negmp = keepp.tile([BL, 1], F32, tag="negmp")    # -max(m_0..m_6)
            ffac = keepp.tile([BL, NSC], F32, tag="ffac")

            # ---------- DMA issue helper (enc tiles stream in order) --------
            enc_tiles = {}

            def issue_tile_dma(t):
                tt = tilep.tile([LT, BL * H], F32, tag="enc_t")
                if t in SLICED:
                    for b in range(BL):
                        nc.sync.dma_start(
                            tt[:, b * H:(b + 1) * H],
                            enc_d[t * LT:(t + 1) * LT, b * H:(b + 1) * H],
                        )
                else:
                    nc.sync.dma_start(tt[:], enc_d[t * LT:(t + 1) * LT, :])
                enc_tiles[t] = tt

            # enc tile 0 first: the stream owns the DMA engines from t~2.4us
            issue_tile_dma(0)

            with (
                tc.tile_pool(name="pre", bufs=1) as prep,
                tc.tile_pool(name="prepsum", bufs=1, space="PSUM") as prepsum,
            ):
                # W + hid right after the first enc tile
                hid_sb = prep.tile([BL, H], F32, tag="hid_sb")
                nc.sync.dma_start(hid_sb[:], hid_d[:])
                w_sb = prep.tile([128, 4 * H], F32, tag="w_sb")
                for j in range(4):
                    nc.sync.dma_start(w_sb[:, j * H:(j + 1) * H], w_d[j * 128:(j + 1) * 128, :])
                issue_tile_dma(1)
                issue_tile_dma(2)

                masks.make_identity(nc, ident[:])

                # h8t[p, j*8+b] = hid[b, j*128+p]
                h8t = prep.tile([128, 4 * BL], F32, tag="h8t")
                for j in range(4):
                    ps = prepsum.tile([128, BL], F32, tag="pre_ps")
                    nc.tensor.transpose(ps[:], hid_sb[:, j * 128:(j + 1) * 128], ident[:BL, :BL])
                    nc.vector.tensor_copy(h8t[:, j * BL:(j + 1) * BL], ps[:])

                # v8[o, j*8+b] = v[b, j*128+o],  v_b = W^T h_b
                v8 = prep.tile([128, 4 * BL], F32, tag="v8")
                for j in range(4):
                    psv = prepsum.tile([128, BL], F32, tag="pre_ps")
                    for i in range(4):
                        nc.tensor.matmul(
                            psv[:],
                            w_sb[:, i * H + j * 128: i * H + (j + 1) * 128],
                            h8t[:, i * BL:(i + 1) * BL],
                            start=(i == 0), stop=(i == 3),
                        )
                    nc.vector.tensor_copy(v8[:, j * BL:(j + 1) * BL], psv[:])

                # vb[p, b*H+h] = v[b, h] for all p. Per-column PE transposes
                # land each v_b row on partition 0 (GPSIMD ISA inputs must
                # start there); ACT evacuates; Pool broadcasts. All on idle
                # preamble slack -- no DMA-stream bytes spent.
                for b in range(BL):
                    psvt = prepsum.tile([1, H], F32, tag="pre_vt")
                    for j in range(4):
                        nc.tensor.transpose(
                            psvt[:, j * 128:(j + 1) * 128],
                            v8[:, j * BL + b: j * BL + b + 1],
                            ident[:],
                        )
                    vt_b = prep.tile([1, H], F32, tag=f"vt{b}")
                    nc.scalar.copy(vt_b[:], psvt[:])
                    nc.gpsimd.partition_broadcast(vb[:, b * H:(b + 1) * H], vt_b[:])

                # masked V blocks for the PE-assist path: block (b,j) is
                # [128,8] with column b = v8[:, j*8+b], rest zero.
                nc.vector.memset(vm[:], 0.0)
                for b in range(3):
                    for j in range(4):
                        blk = (b * 4 + j) * BL
                        nc.vector.tensor_copy(
                            vm[:, blk + b: blk + b + 1],
                            v8[:, j * BL + b: j * BL + b + 1],
                        )

            # ---------- main tile loop ----------
            next_issue = [3]
            premult = []  # big chunks awaiting their exp(m_c - m_prev) factor
            for t in range(NT):
                while next_issue[0] < min(t + 8, NT):
                    issue_tile_dma(next_issue[0])
                    next_issue[0] += 1
                # fold the combine factor into big slices on idle DVE slack
                for _ in range(2):
                    if t >= NT - 4 and premult:
                        cb = premult.pop(0)
                        nc.vector.tensor_scalar_mul(
                            esm[:, cb * LC:(cb + 1) * LC],
                            esm[:, cb * LC:(cb + 1) * LC],
                            ffac[:, cb:cb + 1])
                tt = enc_tiles.pop(t)
                peb = PEB.get(t, [])
                dvb = [b for b in range(BL) if b not in peb]

                eacc = eaccp.tile([LT, BL], F32, tag="eacc")
                if peb:
                    nc.gpsimd.memset(eacc[:, 0:len(peb)], 0.0)
                for b in dvb:
                    nc.vector.scalar_tensor_tensor(
                        out=tt[:, b * H:(b + 1) * H],
                        in0=tt[:, b * H:(b + 1) * H],
                        scalar=1.0,
                        in1=vb[:, b * H:(b + 1) * H],
                        op0=A.mult,
                        op1=A.mult,
                        accum_out=eacc[:, b:b + 1],
                    )
                pe_t = epsum.tile([BL, LT], F32, tag="pe_t")
                if peb:
                    # PE path for b0..2: transpose enc blocks, masked matmuls
                    # accumulate energies into pe_t rows 0..2; the eacc
                    # transpose adds rows 3..7 (cols 0..2 zeroed above).
                    encts = {}
                    for b in peb:
                        tp = tpsum.tile([128, H], F32, tag="tps")
                        for j in range(4):
                            nc.tensor.transpose(
                                tp[:, j * 128:(j + 1) * 128],
                                tt[:, b * H + j * 128: b * H + (j + 1) * 128],
                                ident[:],
                            )
                        enct = enctp.tile([128, H], F32, tag="enct")
                        nc.scalar.copy(enct[:], tp[:])
                        encts[b] = enct
                    first = True
                    for b in peb:
                        for j in range(4):
                            blk = (b * 4 + j) * BL
                            nc.tensor.matmul(
                                pe_t[:],
                                vm[:, blk:blk + BL],
                                encts[b][:, j * 128:(j + 1) * 128],
                                start=first, stop=False,
                            )
                            first = False
                    nc.tensor.matmul(
                        pe_t[:], eacc[:], ident[:],
                        is_transpose=True, start=False, stop=True)
                else:
                    nc.tensor.transpose(pe_t[:], eacc[:], ident[:])
                base = t * LT
                ci = tile_ends_chunk.get(t)
                if ci is not None and ci >= NBIG:
                    # tail tile-chunk: fused PSUM-evac + exp(x - m_prev),
                    # accumulating s_c — one ACT op instead of copy+exp
                    nc.scalar.activation(
                        out=esm[:, base:base + LT], in_=pe_t[:],
                        func=mybir.ActivationFunctionType.Exp,
                        bias=negmp[:], scale=1.0, accum_out=sstore[:, ci:ci + 1])
                    continue
                nc.scalar.copy(esm[:, base:base + LT], pe_t[:])

                if ci is None:
                    continue
                sl = esm[:, SCH[ci][0]:SCH[ci][0] + SCH[ci][1]]
                if ci < NBIG:
                    # big chunk: shift by own max
                    mx_c = smtp.tile([BL, 1], F32, tag=f"mx_{ci % 2}")
                    nc.vector.tensor_reduce(
                        out=mx_c[:], in_=sl, axis=mybir.AxisListType.X, op=A.max)
                    nc.vector.tensor_copy(mstore[:, ci:ci + 1], mx_c[:])
                    negm = smtp.tile([BL, 1], F32, tag=f"ng_{ci % 2}")
                    nc.vector.tensor_scalar_mul(negm[:], mx_c[:], -1.0)
                    nc.scalar.activation(
                        out=sl, in_=sl,
                        func=mybir.ActivationFunctionType.Exp,
                        bias=negm[:], scale=1.0, accum_out=sstore[:, ci:ci + 1])
                    if ci == NBIG - 1:
                        # m_prev = max over big-chunk maxes; precompute the
                        # big-chunk combine factors exp(m_c - m_prev)
                        nc.vector.tensor_reduce(
                            out=negmp[:], in_=mstore[:],
                            axis=mybir.AxisListType.X, op=A.max)
                        nc.vector.tensor_scalar_mul(negmp[:], negmp[:], -1.0)
                        nc.vector.memset(ffac[:, NBIG:], 1.0)
                        nc.scalar.activation(
                            out=ffac[:, 0:NBIG], in_=mstore[:],
                            func=mybir.ActivationFunctionType.Exp,
                            bias=negmp[:], scale=1.0)
                        premult.extend(range(NBIG))

            # ---------- epilogue: S = sum_c s_c * ffac_c; x 1/S; store ------
            # big slices already carry exp(m_c - m_prev); the only remaining
            # factor is the global 1/S, split into three engine regions. The
            # DVE/ACT regions cover big chunks (written long before), so they
            # are gated only by 1/S, not by the last tile's exp.
            ssum = postp.tile([BL, 1], F32, tag="ssum")
            ssum2 = postp.tile([BL, 1], F32, tag="ssum2")
            rsum = postp.tile([BL, 1], F32, tag="rsum")
            wfac = postp.tile([BL, NSC], F32, tag="wfac")
            nc.vector.scalar_tensor_tensor(
                out=wfac[:], in0=ffac[:], scalar=1.0, in1=sstore[:],
                op0=A.mult, op1=A.mult, accum_out=ssum[:])
            nc.vector.tensor_copy(ssum2[:], ssum[:])
            nc.vector.reciprocal(rsum[:], ssum[:])

            # x 1/S in three engine regions (Pool divides internally)
            D0, D1 = 2240, 3328
            nc.vector.tensor_scalar_mul(esm[:, 0:D0], esm[:, 0:D0], rsum[:])
            nc.scalar.activation(
                out=esm[:, D0:D1], in_=esm[:, D0:D1],
                func=mybir.ActivationFunctionType.Copy, scale=rsum[:])
            nc.gpsimd.normalize_recip(esm[:, D1:], esm[:, D1:], ssum2[:])
            nc.sync.dma_start(out_d[:], esm[:])

    if do_compile:
        nc.compile()
    return nc


def kernel(hidden, encoder_outputs, W, b):
    hidden = np.asarray(hidden, dtype=np.float32)
    enc = np.asarray(encoder_outputs, dtype=np.float32)
    W = np.asarray(W, dtype=np.float32)

    if "nc" not in _cache:
        _cache["nc"] = _build()
    nc = _cache["nc"]

    in_maps = []
    for c in range(NCORES):
        b0 = c * BL
        in_maps.append({
            "enc": np.ascontiguousarray(enc[:, b0:b0 + BL, :]).reshape(L, BL * H),
            "hid": np.ascontiguousarray(hidden[0, b0:b0 + BL, :]),
            "w": W,
        })
    res = run_bass_kernel_spmd(nc, in_maps, core_ids=list(range(NCORES)))
    out = np.empty((B, 1, L), dtype=np.float32)
    for c in range(NCORES):
        out[c * BL:(c + 1) * BL, 0, :] = res.results[c]["out"]
    return out
